# revision 18
# baseline (speedup 1.0000x reference)
"""Batched min-variance weights kernel for Trainium2.

w = S^-1 1 / (1^T S^-1 1) for 8192 SPD 64x64 matrices, data-parallel over
8 cores (1024 matrices each). Measured ~0.61 ms on-device vs the 2.19 ms
v2 baseline.

Design:
- Full-interleave fp16 layout: each SBUF partition lane holds its 8
  matrices element-interleaved — element (i, 8*j+g) is matrix g's (i,j).
  With 2-byte elements every rank-1-update operand has innermost stride
  +1 over the G=8 group and 16-byte-aligned row starts, so the DVE's
  2x_1P fp16 perf mode engages on both the outer-product multiply and
  the subtract (hardware-verified 0.52 ns/elem vs 1.04 for fp32), AND
  each elimination step needs only ~O(bands) instructions for all 1024
  matrices (the v2 baseline issued ~4500 tiny per-tile ops and was ~80%
  per-instruction overhead).
- fp32->fp16 conversion + interleaving happen on HOST (numpy): the
  device reads a pre-interleaved fp16 tensor (on-device CAST is slow,
  and this also halves the host->device transfer).
- Symmetric elimination updates a staircase of row-bands covering the
  lower triangle + the bordered rhs row (row 64); the band count per
  step (up to ~7) minimizes measured fixed-cost (152 ns/op) vs streamed
  area. Overspill above the diagonal is harmless (dead data). Input DMA
  is chunked to step-0's first bands so compute fully hides the load.
- Back-substitution: diagonal reciprocals in ONE op after the forward
  pass, then per step a tiny q = Rd_i*yhat_i factor and 2 folded ops;
  normalization and de-interleave fuse into one op.
- Precision: fp16 storage, reciprocals rounded to fp16 (numpy-validated;
  measured rel err 9.9e-3 full-batch vs the 2e-2 tolerance).
- GpSimd deliberately unused: concurrent pool ops slow DVE 4.3x via the
  shared SBUF port (hardware-measured), so DVE-only is strictly faster.
"""

import os

import numpy as np

B = 8192
N = 64
H = N + 1          # 64 matrix rows + rhs row
G = 8              # matrices interleaved per lane
W = G * N          # interleaved row width (512)
NCORES = 8
BPC = B // NCORES
P = 128

_CACHE = {}
LAST_EXEC_NS = None


def _patch_tail_drain():
    import concourse.mybir as mybir
    import concourse.tile as tile_mod
    from concourse.bass import SemaphoreHandle
    from concourse.vector_clock import ScopedClock

    if getattr(tile_mod.TileContext, "_drain_split_patched", False):
        return

    def _drain_and_barrier(self, tick_clock, wait_clock):
        drain_inst = self.nc.sync.drain()
        wait_clock.add_sem_waits(
            drain_inst.ins, ScopedClock({None: tick_clock.global_clock})
        )
        si = drain_inst.ins.sync_info
        if si is not None and len(si.on_wait) > 1:
            waits = list(si.on_wait)
            drain_inst.ins.sync_info = mybir.SyncInfo(
                on_wait=[waits[0]], on_update=list(si.on_update)
            )
            for w in waits[1:]:
                self.nc.sync.wait_ge(
                    SemaphoreHandle(w.ant_name, w.id), w.wait_value
                )
        self.nc.all_engine_barrier()
        assert self.sems is not None
        popped = self.nc._tile_sem_poison_stack.pop()
        assert popped is self._sem_poison
        self.nc.clear_and_free_semaphores(list(self.sems.allocated().values()))
        self.nc.all_engine_barrier()

    tile_mod.TileContext._drain_and_barrier = _drain_and_barrier
    tile_mod.TileContext._drain_split_patched = True


_BAND_FIXED_NS = 310.0        # measured cost of an extra (mult+sub) pair
_BAND_RATE_NS = G * 2 * 0.52  # ns per staircase area-unit (both passes)


def _bands(lo):
    """Row-bands covering {(i,j): lo<=i<=64, lo<=j<=min(i,63)} (row 64 is
    the bordered rhs row, needing cols lo..63). Band rows [r0,r1) update
    cols [lo, c1); overspill above the diagonal is dead data. The band
    count minimizes stream-vs-instruction-overhead cost; band index
    selects the PT scratch tile (sized for the worst case)."""
    best = None
    for nb in range(1, 10):
        rows = H - lo
        bounds = [lo + (rows * i) // nb for i in range(nb + 1)]
        bl = []
        area = 0
        for i in range(nb):
            r0, r1 = bounds[i], bounds[i + 1]
            if r1 <= r0:
                continue
            c1 = min(r1 - 1, N - 1) + 1
            area += (r1 - r0) * (c1 - lo)
            bl.append((r0, r1, lo, c1))
        cost = area * _BAND_RATE_NS + len(bl) * _BAND_FIXED_NS
        if best is None or cost < best[0]:
            best = (cost, bl)
    return [(i, r0, r1, c0, c1) for i, (r0, r1, c0, c1) in enumerate(best[1])]


def _pt_dims():
    """Max (rows, cols) per band index across all steps."""
    md = {}
    for k in range(N - 1):
        for (i, r0, r1, c0, c1) in _bands(k + 1):
            mr, mc = md.get(i, (0, 0))
            md[i] = (max(mr, r1 - r0), max(mc, c1 - c0))
    return md


def _build_program():
    import concourse.bass as bass
    import concourse.mybir as mybir
    from concourse.tile import TileContext

    _patch_tail_drain()

    fp32 = mybir.dt.float32
    fp16 = mybir.dt.float16
    Alu = mybir.AluOpType

    nc = bass.Bass()
    sig16 = nc.dram_tensor("sig16", [P, H, W], fp16, kind="ExternalInput")
    wout = nc.dram_tensor("w", [P, G, N], fp32, kind="ExternalOutput")

    with TileContext(nc) as tc:
        with (
            tc.tile_pool(name="mpool", bufs=1) as mpool,
            tc.tile_pool(name="ptpool", bufs=1) as ptpool,
            tc.tile_pool(name="zpool", bufs=1) as zpool,
        ):
            X2 = mpool.tile([P, H, W], fp16, tag="X2")
            PTS = []
            for i, (mr, mc) in sorted(_pt_dims().items()):
                PTi = ptpool.tile(
                    [P, mr, G * mc], fp16, tag=f"PT{i}", name=f"PT{i}"
                )
                PTS.append(PTi)
            TMP = zpool.tile([P, N - 1, G], fp16, tag="TMP")
            Rdf = zpool.tile([P, N, G], fp32, tag="Rdf")
            # fixed per-step scratch (same-engine reuse needs no semaphores)
            R16 = zpool.tile([P, G], fp16, tag="R16")
            QF = zpool.tile([P, G], fp16, tag="QF")
            scol = zpool.tile([P, N, G], fp16, tag="scol")
            ZF = zpool.tile([P, W], fp32, tag="ZF")
            S2 = zpool.tile([P, G, 1], fp32, tag="S2")
            RS = zpool.tile([P, G, 1], fp32, tag="RS")
            WV = zpool.tile([P, G, N], fp32, tag="WV")

            # Row-chunk DMAs aligned to step-0's bands (parallel queues):
            # step 0 can start on chunk 0 before chunks 1/2 land. One
            # touch per chunk so each instruction carries one sem wait.
            rsplit = [0, 22, 43, H]
            for ci in range(3):
                nc.sync.dma_start(
                    out=X2[:, rsplit[ci]:rsplit[ci + 1], :],
                    in_=sig16[:, rsplit[ci]:rsplit[ci + 1], :],
                )

            def _touch(ci):
                r = rsplit[ci]
                nc.vector.tensor_copy(X2[:, r, 0:2], X2[:, r, 0:2])

            # ---- forward: symmetric GE on interleaved lower + rhs row ----
            for k in range(N - 1):
                lo = k + 1
                piv = X2[:, k, G * k:G * k + G]
                if k == 0:
                    _touch(0)
                # fp16 output rounds the internally-fp32 reciprocal — same
                # numerics as recip-to-fp32 followed by a cast (validated
                # against the fp32-reciprocal numpy sim: rel err 8.1e-3).
                with nc.allow_low_precision(
                    reason="fp16 recip == fp32 recip + fp16 cast"
                ):
                    nc.vector.reciprocal(R16[:, :], piv)

                def _scol(a, b):
                    # scol[r, g] = X2[lo+r, G*k+g] * R16[g], rows a..b-1
                    nc.vector.tensor_tensor(
                        out=scol[:, a - lo:b - lo, :],
                        in0=X2[:, a:b, G * k:G * k + G],
                        in1=R16[:, :].unsqueeze(1).broadcast_to(
                            [P, b - a, G]
                        ),
                        op=Alu.mult,
                    )

                if k > 0:
                    _scol(lo, H)
                else:
                    touched = 1  # chunk 0 touched before the reciprocal
                for (pi, r0, r1, c0, c1) in _bands(lo):
                    if k == 0:
                        # touch chunks as step-0 bands first reach them
                        while touched < 3 and r1 > rsplit[touched]:
                            _touch(touched)
                            touched += 1
                        _scol(r0, r1)
                    nr = r1 - r0
                    ncc = c1 - c0
                    PT = PTS[pi]
                    # PT[r, j, g] = scol[r, g] * X2[j(as row), G*k+g]
                    nc.vector.tensor_tensor(
                        out=PT[:, 0:nr, 0:G * ncc].rearrange(
                            "p a (b c) -> p a b c", c=G
                        ),
                        in0=scol[:, r0 - lo:r1 - lo, :]
                        .unsqueeze(2)
                        .broadcast_to([P, nr, ncc, G]),
                        in1=X2[:, c0:c1, G * k:G * k + G]
                        .unsqueeze(1)
                        .broadcast_to([P, nr, ncc, G]),
                        op=Alu.mult,
                    )
                    nc.vector.tensor_tensor(
                        out=X2[:, r0:r1, G * c0:G * c1],
                        in0=X2[:, r0:r1, G * c0:G * c1],
                        in1=PT[:, 0:nr, 0:G * ncc],
                        op=Alu.subtract,
                    )

            # ---- diag reciprocals (one op) ----
            diagv = X2[:, 0:N, :].rearrange(
                "p a (b c) -> p (a b) c", c=G
            )[:, 0:N * N:N + 1, :]
            nc.vector.reciprocal(Rdf[:, :, :], diagv)

            # ---- back-substitution on rhs row (yhat recurrence) ----
            # yhat[0:i] -= row_i[0:i] * (Rd_i * yhat_i)
            for i in range(N - 1, 0, -1):
                q = QF
                with nc.allow_low_precision(reason="fp16 backsolve factor"):
                    nc.vector.tensor_tensor(
                        out=q[:, :],
                        in0=X2[:, H - 1, G * i:G * i + G],
                        in1=Rdf[:, i, :],
                        op=Alu.mult,
                    )
                nc.vector.tensor_tensor(
                    out=TMP[:, 0:i, :],
                    in0=X2[:, i, 0:G * i].rearrange("p (a b) -> p a b", b=G),
                    in1=q[:, :].unsqueeze(1).broadcast_to([P, i, G]),
                    op=Alu.mult,
                )
                nc.vector.tensor_tensor(
                    out=X2[:, H - 1, 0:G * i],
                    in0=X2[:, H - 1, 0:G * i],
                    in1=TMP[:, 0:i, :].rearrange("p a b -> p (a b)"),
                    op=Alu.subtract,
                )

            # ---- z = yhat * Rd (sign cancels in normalization) ----
            nc.vector.tensor_tensor(
                out=ZF[:, :],
                in0=X2[:, H - 1, :],
                in1=Rdf[:, :, :].rearrange("p a b -> p (a b)"),
                op=Alu.mult,
            )
            # per-matrix sums: view [P, G, 64] (stride 1 over g) and
            # reduce the innermost j axis
            nc.vector.tensor_reduce(
                out=S2[:, :, :],
                in_=ZF[:, :].rearrange("p (a b) -> p b a", b=G),
                axis=mybir.AxisListType.X,
                op=Alu.add,
            )
            nc.vector.reciprocal(RS[:, :, :], S2[:, :, :])
            # w = z / sum, de-interleaved to [g, n]
            nc.vector.tensor_tensor(
                out=WV[:, :, :],
                in0=ZF[:, :].rearrange("p (a b) -> p b a", b=G),
                in1=RS[:, :, :].broadcast_to([P, G, N]),
                op=Alu.mult,
            )
            nc.sync.dma_start(out=wout[:, :, :], in_=WV[:, :, :])

    return nc


def _prep_core(shard):
    """[1024, 64, 64] fp32 -> [P, H, W] fp16 interleaved."""
    til = shard.reshape(G, P, N, N).astype(np.float16)
    arr = np.empty((P, H, W), dtype=np.float16)
    arr[:, :N, :].reshape(P, N, N, G)[...] = til.transpose(1, 2, 3, 0)
    arr[:, N, :] = np.float16(1.0)
    return arr


def _unprep_core(warr):
    """[P, G, N] fp32 -> [1024, 64] fp32."""
    return warr.transpose(1, 0, 2).reshape(BPC, N)


def kernel(sigma: np.ndarray) -> np.ndarray:
    global LAST_EXEC_NS
    import time

    from concourse.bass_utils import run_bass_kernel_spmd

    if "nc" not in _CACHE:
        _CACHE["nc"] = _build_program()
    nc = _CACHE["nc"]

    sigma = np.ascontiguousarray(sigma, dtype=np.float32)
    shards = sigma.reshape(NCORES, BPC, N, N)
    in_maps = [{"sig16": _prep_core(shards[i])} for i in range(NCORES)]

    res = run_bass_kernel_spmd(nc, in_maps, core_ids=list(range(NCORES)))

    if os.environ.get("BASS_KERNEL_TIME", "0") == "1":
        # On-device NEFF time via neuron-profile when available; wall time
        # of a warm run otherwise.
        exec_ns = None
        try:
            exec_ns, _ = profile_exec_ns(sigma)
        except Exception:
            exec_ns = None
        if exec_ns is None:
            t0 = time.perf_counter()
            res = run_bass_kernel_spmd(
                nc, in_maps, core_ids=list(range(NCORES))
            )
            exec_ns = int((time.perf_counter() - t0) * 1e9)
        LAST_EXEC_NS = exec_ns

    out = np.concatenate(
        [_unprep_core(res.results[i]["w"]) for i in range(NCORES)], axis=0
    )
    return out.reshape(B, N, 1).astype(np.float32)


def profile_exec_ns(sigma: np.ndarray, tmpdir: str | None = None):
    """Run once with NTFF tracing; returns (exec_time_ns, output)."""
    from concourse.bass_utils import run_bass_kernel_spmd

    if "nc" not in _CACHE:
        _CACHE["nc"] = _build_program()
    nc = _CACHE["nc"]
    sigma = np.ascontiguousarray(sigma, dtype=np.float32)
    shards = sigma.reshape(NCORES, BPC, N, N)
    in_maps = [{"sig16": _prep_core(shards[i])} for i in range(NCORES)]
    res = run_bass_kernel_spmd(
        nc, in_maps, core_ids=list(range(NCORES)), trace=True, tmpdir=tmpdir
    )
    out = np.concatenate(
        [_unprep_core(res.results[i]["w"]) for i in range(NCORES)], axis=0
    )
    return res.exec_time_ns, out.reshape(B, N, 1).astype(np.float32)


# revision 20
# speedup vs baseline: 1.0116x; 1.0116x over previous
"""Batched min-variance weights kernel for Trainium2.

w = S^-1 1 / (1^T S^-1 1) for 8192 SPD 64x64 matrices, data-parallel over
8 cores (1024 matrices each). Measured ~0.61 ms on-device vs the 2.19 ms
v2 baseline.

Design:
- Full-interleave fp16 layout: each SBUF partition lane holds its 8
  matrices element-interleaved — element (i, 8*j+g) is matrix g's (i,j).
  With 2-byte elements every rank-1-update operand has innermost stride
  +1 over the G=8 group and 16-byte-aligned row starts, so the DVE's
  2x_1P fp16 perf mode engages on both the outer-product multiply and
  the subtract (hardware-verified 0.52 ns/elem vs 1.04 for fp32), AND
  each elimination step needs only ~O(bands) instructions for all 1024
  matrices (the v2 baseline issued ~4500 tiny per-tile ops and was ~80%
  per-instruction overhead).
- fp32->fp16 conversion + interleaving happen on HOST (numpy): the
  device reads a pre-interleaved fp16 tensor (on-device CAST is slow,
  and this also halves the host->device transfer).
- Symmetric elimination updates a staircase of row-bands covering the
  lower triangle + the bordered rhs row (row 64); the band count per
  step (up to ~7) minimizes measured fixed-cost (152 ns/op) vs streamed
  area. Overspill above the diagonal is harmless (dead data). Input DMA
  is chunked to step-0's first bands so compute fully hides the load.
- Back-substitution: diagonal reciprocals in ONE op after the forward
  pass, then per step a tiny q = Rd_i*yhat_i factor and 2 folded ops;
  normalization and de-interleave fuse into one op.
- Precision: fp16 storage, reciprocals rounded to fp16 (numpy-validated;
  measured rel err 9.9e-3 full-batch vs the 2e-2 tolerance).
- GpSimd deliberately unused: concurrent pool ops slow DVE 4.3x via the
  shared SBUF port (hardware-measured), so DVE-only is strictly faster.
"""

import os

import numpy as np

B = 8192
N = 64
H = N + 1          # 64 matrix rows + rhs row
G = 8              # matrices interleaved per lane
W = G * N          # interleaved row width (512)
NCORES = 8
BPC = B // NCORES
P = 128

_CACHE = {}
LAST_EXEC_NS = None


def _patch_tail_drain():
    import concourse.mybir as mybir
    import concourse.tile as tile_mod
    from concourse.bass import SemaphoreHandle
    from concourse.vector_clock import ScopedClock

    if getattr(tile_mod.TileContext, "_drain_split_patched", False):
        return

    def _drain_and_barrier(self, tick_clock, wait_clock):
        drain_inst = self.nc.sync.drain()
        wait_clock.add_sem_waits(
            drain_inst.ins, ScopedClock({None: tick_clock.global_clock})
        )
        si = drain_inst.ins.sync_info
        if si is not None and len(si.on_wait) > 1:
            waits = list(si.on_wait)
            drain_inst.ins.sync_info = mybir.SyncInfo(
                on_wait=[waits[0]], on_update=list(si.on_update)
            )
            for w in waits[1:]:
                self.nc.sync.wait_ge(
                    SemaphoreHandle(w.ant_name, w.id), w.wait_value
                )
        self.nc.all_engine_barrier()
        assert self.sems is not None
        popped = self.nc._tile_sem_poison_stack.pop()
        assert popped is self._sem_poison
        self.nc.clear_and_free_semaphores(list(self.sems.allocated().values()))
        self.nc.all_engine_barrier()

    tile_mod.TileContext._drain_and_barrier = _drain_and_barrier
    tile_mod.TileContext._drain_split_patched = True


_BAND_FIXED_NS = 310.0        # measured cost of an extra (mult+sub) pair
_BAND_RATE_NS = G * 2 * 0.52  # ns per staircase area-unit (both passes)


def _bands(lo):
    """Row-bands covering {(i,j): lo<=i<=64, lo<=j<=min(i,63)} (row 64 is
    the bordered rhs row, needing cols lo..63). Band rows [r0,r1) update
    cols [lo, c1); overspill above the diagonal is dead data. The band
    count minimizes stream-vs-instruction-overhead cost; band index
    selects the PT scratch tile (sized for the worst case)."""
    best = None
    for nb in range(1, 10):
        rows = H - lo
        bounds = [lo + (rows * i) // nb for i in range(nb + 1)]
        bl = []
        area = 0
        for i in range(nb):
            r0, r1 = bounds[i], bounds[i + 1]
            if r1 <= r0:
                continue
            c1 = min(r1 - 1, N - 1) + 1
            area += (r1 - r0) * (c1 - lo)
            bl.append((r0, r1, lo, c1))
        cost = area * _BAND_RATE_NS + len(bl) * _BAND_FIXED_NS
        if best is None or cost < best[0]:
            best = (cost, bl)
    return [(i, r0, r1, c0, c1) for i, (r0, r1, c0, c1) in enumerate(best[1])]


def _pt_dims():
    """Max (rows, cols) per band index across all steps."""
    md = {}
    for k in range(N - 1):
        for (i, r0, r1, c0, c1) in _bands(k + 1):
            mr, mc = md.get(i, (0, 0))
            md[i] = (max(mr, r1 - r0), max(mc, c1 - c0))
    return md


def _build_program():
    import concourse.bass as bass
    import concourse.mybir as mybir
    from concourse.tile import TileContext

    _patch_tail_drain()

    fp32 = mybir.dt.float32
    fp16 = mybir.dt.float16
    Alu = mybir.AluOpType

    nc = bass.Bass()
    sig16 = nc.dram_tensor("sig16", [P, H, W], fp16, kind="ExternalInput")
    wout = nc.dram_tensor("w", [P, G, N], fp32, kind="ExternalOutput")

    with TileContext(nc) as tc:
        with (
            tc.tile_pool(name="mpool", bufs=1) as mpool,
            tc.tile_pool(name="ptpool", bufs=1) as ptpool,
            tc.tile_pool(name="zpool", bufs=1) as zpool,
        ):
            X2 = mpool.tile([P, H, W], fp16, tag="X2")
            PTS = []
            for i, (mr, mc) in sorted(_pt_dims().items()):
                PTi = ptpool.tile(
                    [P, mr, G * mc], fp16, tag=f"PT{i}", name=f"PT{i}"
                )
                PTS.append(PTi)
            TMP = zpool.tile([P, N - 1, G], fp16, tag="TMP")
            Rdf = zpool.tile([P, N, G], fp32, tag="Rdf")
            # fixed per-step scratch (same-engine reuse needs no semaphores)
            R16 = zpool.tile([P, G], fp16, tag="R16")
            QF = zpool.tile([P, G], fp16, tag="QF")
            scol = zpool.tile([P, N, G], fp16, tag="scol")
            ZF = zpool.tile([P, W], fp32, tag="ZF")
            S2 = zpool.tile([P, G, 1], fp32, tag="S2")
            RS = zpool.tile([P, G, 1], fp32, tag="RS")
            WV = zpool.tile([P, G, N], fp32, tag="WV")

            # Row-chunk DMAs aligned exactly to step-0's band boundaries:
            # band d of step 0 touches only rows < its r1, so compute can
            # start as soon as the first ~10-row chunk lands and the rest
            # of the load pipelines behind the step-0 band updates. One
            # touch per chunk so each instruction carries one sem wait.
            rsplit = [0] + [r1 for (_, r0, r1, c0, c1) in _bands(1)]
            nchunks = len(rsplit) - 1
            for ci in range(nchunks):
                nc.sync.dma_start(
                    out=X2[:, rsplit[ci]:rsplit[ci + 1], :],
                    in_=sig16[:, rsplit[ci]:rsplit[ci + 1], :],
                )

            def _touch(ci):
                r = rsplit[ci]
                nc.vector.tensor_copy(X2[:, r, 0:2], X2[:, r, 0:2])

            # ---- forward: symmetric GE on interleaved lower + rhs row ----
            for k in range(N - 1):
                lo = k + 1
                piv = X2[:, k, G * k:G * k + G]
                if k == 0:
                    _touch(0)
                # fp16 output rounds the internally-fp32 reciprocal — same
                # numerics as recip-to-fp32 followed by a cast (validated
                # against the fp32-reciprocal numpy sim: rel err 8.1e-3).
                with nc.allow_low_precision(
                    reason="fp16 recip == fp32 recip + fp16 cast"
                ):
                    nc.vector.reciprocal(R16[:, :], piv)

                def _scol(a, b):
                    # scol[r, g] = X2[lo+r, G*k+g] * R16[g], rows a..b-1
                    nc.vector.tensor_tensor(
                        out=scol[:, a - lo:b - lo, :],
                        in0=X2[:, a:b, G * k:G * k + G],
                        in1=R16[:, :].unsqueeze(1).broadcast_to(
                            [P, b - a, G]
                        ),
                        op=Alu.mult,
                    )

                if k > 0:
                    _scol(lo, H)
                else:
                    touched = 1  # chunk 0 touched before the reciprocal
                for (pi, r0, r1, c0, c1) in _bands(lo):
                    if k == 0:
                        # touch chunks as step-0 bands first reach them
                        while touched < nchunks and r1 > rsplit[touched]:
                            _touch(touched)
                            touched += 1
                        _scol(r0, r1)
                    nr = r1 - r0
                    ncc = c1 - c0
                    PT = PTS[pi]
                    # PT[r, j, g] = scol[r, g] * X2[j(as row), G*k+g]
                    nc.vector.tensor_tensor(
                        out=PT[:, 0:nr, 0:G * ncc].rearrange(
                            "p a (b c) -> p a b c", c=G
                        ),
                        in0=scol[:, r0 - lo:r1 - lo, :]
                        .unsqueeze(2)
                        .broadcast_to([P, nr, ncc, G]),
                        in1=X2[:, c0:c1, G * k:G * k + G]
                        .unsqueeze(1)
                        .broadcast_to([P, nr, ncc, G]),
                        op=Alu.mult,
                    )
                    nc.vector.tensor_tensor(
                        out=X2[:, r0:r1, G * c0:G * c1],
                        in0=X2[:, r0:r1, G * c0:G * c1],
                        in1=PT[:, 0:nr, 0:G * ncc],
                        op=Alu.subtract,
                    )

            # ---- diag reciprocals (one op) ----
            diagv = X2[:, 0:N, :].rearrange(
                "p a (b c) -> p (a b) c", c=G
            )[:, 0:N * N:N + 1, :]
            nc.vector.reciprocal(Rdf[:, :, :], diagv)

            # ---- back-substitution on rhs row (yhat recurrence) ----
            # yhat[0:i] -= row_i[0:i] * (Rd_i * yhat_i)
            for i in range(N - 1, 0, -1):
                q = QF
                with nc.allow_low_precision(reason="fp16 backsolve factor"):
                    nc.vector.tensor_tensor(
                        out=q[:, :],
                        in0=X2[:, H - 1, G * i:G * i + G],
                        in1=Rdf[:, i, :],
                        op=Alu.mult,
                    )
                nc.vector.tensor_tensor(
                    out=TMP[:, 0:i, :],
                    in0=X2[:, i, 0:G * i].rearrange("p (a b) -> p a b", b=G),
                    in1=q[:, :].unsqueeze(1).broadcast_to([P, i, G]),
                    op=Alu.mult,
                )
                nc.vector.tensor_tensor(
                    out=X2[:, H - 1, 0:G * i],
                    in0=X2[:, H - 1, 0:G * i],
                    in1=TMP[:, 0:i, :].rearrange("p a b -> p (a b)"),
                    op=Alu.subtract,
                )

            # ---- z = yhat * Rd (sign cancels in normalization) ----
            nc.vector.tensor_tensor(
                out=ZF[:, :],
                in0=X2[:, H - 1, :],
                in1=Rdf[:, :, :].rearrange("p a b -> p (a b)"),
                op=Alu.mult,
            )
            # per-matrix sums: view [P, G, 64] (stride 1 over g) and
            # reduce the innermost j axis
            nc.vector.tensor_reduce(
                out=S2[:, :, :],
                in_=ZF[:, :].rearrange("p (a b) -> p b a", b=G),
                axis=mybir.AxisListType.X,
                op=Alu.add,
            )
            nc.vector.reciprocal(RS[:, :, :], S2[:, :, :])
            # w = z / sum, de-interleaved to [g, n]
            nc.vector.tensor_tensor(
                out=WV[:, :, :],
                in0=ZF[:, :].rearrange("p (a b) -> p b a", b=G),
                in1=RS[:, :, :].broadcast_to([P, G, N]),
                op=Alu.mult,
            )
            nc.sync.dma_start(out=wout[:, :, :], in_=WV[:, :, :])

    return nc


def _prep_core(shard):
    """[1024, 64, 64] fp32 -> [P, H, W] fp16 interleaved."""
    til = shard.reshape(G, P, N, N).astype(np.float16)
    arr = np.empty((P, H, W), dtype=np.float16)
    arr[:, :N, :].reshape(P, N, N, G)[...] = til.transpose(1, 2, 3, 0)
    arr[:, N, :] = np.float16(1.0)
    return arr


def _unprep_core(warr):
    """[P, G, N] fp32 -> [1024, 64] fp32."""
    return warr.transpose(1, 0, 2).reshape(BPC, N)


def kernel(sigma: np.ndarray) -> np.ndarray:
    global LAST_EXEC_NS
    import time

    from concourse.bass_utils import run_bass_kernel_spmd

    if "nc" not in _CACHE:
        _CACHE["nc"] = _build_program()
    nc = _CACHE["nc"]

    sigma = np.ascontiguousarray(sigma, dtype=np.float32)
    shards = sigma.reshape(NCORES, BPC, N, N)
    in_maps = [{"sig16": _prep_core(shards[i])} for i in range(NCORES)]

    res = run_bass_kernel_spmd(nc, in_maps, core_ids=list(range(NCORES)))

    if os.environ.get("BASS_KERNEL_TIME", "0") == "1":
        # On-device NEFF time via neuron-profile when available; wall time
        # of a warm run otherwise.
        exec_ns = None
        try:
            exec_ns, _ = profile_exec_ns(sigma)
        except Exception:
            exec_ns = None
        if exec_ns is None:
            t0 = time.perf_counter()
            res = run_bass_kernel_spmd(
                nc, in_maps, core_ids=list(range(NCORES))
            )
            exec_ns = int((time.perf_counter() - t0) * 1e9)
        LAST_EXEC_NS = exec_ns

    out = np.concatenate(
        [_unprep_core(res.results[i]["w"]) for i in range(NCORES)], axis=0
    )
    return out.reshape(B, N, 1).astype(np.float32)


def profile_exec_ns(sigma: np.ndarray, tmpdir: str | None = None):
    """Run once with NTFF tracing; returns (exec_time_ns, output)."""
    from concourse.bass_utils import run_bass_kernel_spmd

    if "nc" not in _CACHE:
        _CACHE["nc"] = _build_program()
    nc = _CACHE["nc"]
    sigma = np.ascontiguousarray(sigma, dtype=np.float32)
    shards = sigma.reshape(NCORES, BPC, N, N)
    in_maps = [{"sig16": _prep_core(shards[i])} for i in range(NCORES)]
    res = run_bass_kernel_spmd(
        nc, in_maps, core_ids=list(range(NCORES)), trace=True, tmpdir=tmpdir
    )
    out = np.concatenate(
        [_unprep_core(res.results[i]["w"]) for i in range(NCORES)], axis=0
    )
    return res.exec_time_ns, out.reshape(B, N, 1).astype(np.float32)


# revision 21
# speedup vs baseline: 1.0159x; 1.0042x over previous
"""Batched min-variance weights kernel for Trainium2.

w = S^-1 1 / (1^T S^-1 1) for 8192 SPD 64x64 matrices, data-parallel over
8 cores (1024 matrices each). Measured ~0.61 ms on-device vs the 2.19 ms
v2 baseline.

Design:
- Full-interleave fp16 layout: each SBUF partition lane holds its 8
  matrices element-interleaved — element (i, 8*j+g) is matrix g's (i,j).
  With 2-byte elements every rank-1-update operand has innermost stride
  +1 over the G=8 group and 16-byte-aligned row starts, so the DVE's
  2x_1P fp16 perf mode engages on both the outer-product multiply and
  the subtract (hardware-verified 0.52 ns/elem vs 1.04 for fp32), AND
  each elimination step needs only ~O(bands) instructions for all 1024
  matrices (the v2 baseline issued ~4500 tiny per-tile ops and was ~80%
  per-instruction overhead).
- fp32->fp16 conversion + interleaving happen on HOST (numpy): the
  device reads a pre-interleaved fp16 tensor (on-device CAST is slow,
  and this also halves the host->device transfer).
- Symmetric elimination updates a staircase of row-bands covering the
  lower triangle + the bordered rhs row (row 64); the band count per
  step (up to ~7) minimizes measured fixed-cost (152 ns/op) vs streamed
  area. Overspill above the diagonal is harmless (dead data). Input DMA
  is chunked to step-0's first bands so compute fully hides the load.
- Back-substitution: diagonal reciprocals in ONE op after the forward
  pass, then per step a tiny q = Rd_i*yhat_i factor and 2 folded ops;
  normalization and de-interleave fuse into one op.
- Precision: fp16 storage, reciprocals rounded to fp16 (numpy-validated;
  measured rel err 9.9e-3 full-batch vs the 2e-2 tolerance).
- GpSimd deliberately unused: concurrent pool ops slow DVE 4.3x via the
  shared SBUF port (hardware-measured), so DVE-only is strictly faster.
"""

import os

import numpy as np

B = 8192
N = 64
H = N + 1          # 64 matrix rows + rhs row
G = 8              # matrices interleaved per lane
W = G * N          # interleaved row width (512)
NCORES = 8
BPC = B // NCORES
P = 128

_CACHE = {}
LAST_EXEC_NS = None


def _patch_tail_drain():
    import concourse.mybir as mybir
    import concourse.tile as tile_mod
    from concourse.bass import SemaphoreHandle
    from concourse.vector_clock import ScopedClock

    if getattr(tile_mod.TileContext, "_drain_split_patched", False):
        return

    def _drain_and_barrier(self, tick_clock, wait_clock):
        drain_inst = self.nc.sync.drain()
        wait_clock.add_sem_waits(
            drain_inst.ins, ScopedClock({None: tick_clock.global_clock})
        )
        si = drain_inst.ins.sync_info
        if si is not None and len(si.on_wait) > 1:
            waits = list(si.on_wait)
            drain_inst.ins.sync_info = mybir.SyncInfo(
                on_wait=[waits[0]], on_update=list(si.on_update)
            )
            for w in waits[1:]:
                self.nc.sync.wait_ge(
                    SemaphoreHandle(w.ant_name, w.id), w.wait_value
                )
        self.nc.all_engine_barrier()
        assert self.sems is not None
        popped = self.nc._tile_sem_poison_stack.pop()
        assert popped is self._sem_poison
        self.nc.clear_and_free_semaphores(list(self.sems.allocated().values()))
        self.nc.all_engine_barrier()

    tile_mod.TileContext._drain_and_barrier = _drain_and_barrier
    tile_mod.TileContext._drain_split_patched = True


_BAND_FIXED_NS = 310.0        # measured cost of an extra (mult+sub) pair
_BAND_RATE_NS = G * 2 * 0.52  # ns per staircase area-unit (both passes)


def _bands(lo):
    """Row-bands covering {(i,j): lo<=i<=64, lo<=j<=min(i,63)} (row 64 is
    the bordered rhs row, needing cols lo..63). Band rows [r0,r1) update
    cols [lo, c1); overspill above the diagonal is dead data. The band
    count minimizes stream-vs-instruction-overhead cost; band index
    selects the PT scratch tile (sized for the worst case)."""
    best = None
    for nb in range(1, 10):
        rows = H - lo
        bounds = [lo + (rows * i) // nb for i in range(nb + 1)]
        bl = []
        area = 0
        for i in range(nb):
            r0, r1 = bounds[i], bounds[i + 1]
            if r1 <= r0:
                continue
            c1 = min(r1 - 1, N - 1) + 1
            area += (r1 - r0) * (c1 - lo)
            bl.append((r0, r1, lo, c1))
        cost = area * _BAND_RATE_NS + len(bl) * _BAND_FIXED_NS
        if best is None or cost < best[0]:
            best = (cost, bl)
    return [(i, r0, r1, c0, c1) for i, (r0, r1, c0, c1) in enumerate(best[1])]


def _pt_dims():
    """Max (rows, cols) per band index across all steps."""
    md = {}
    for k in range(N - 1):
        for (i, r0, r1, c0, c1) in _bands(k + 1):
            mr, mc = md.get(i, (0, 0))
            md[i] = (max(mr, r1 - r0), max(mc, c1 - c0))
    return md


def _build_program():
    import concourse.bass as bass
    import concourse.mybir as mybir
    from concourse.tile import TileContext

    _patch_tail_drain()

    fp32 = mybir.dt.float32
    fp16 = mybir.dt.float16
    Alu = mybir.AluOpType

    nc = bass.Bass()
    sig16 = nc.dram_tensor("sig16", [P, H, W], fp16, kind="ExternalInput")
    wout = nc.dram_tensor("w", [P, G, N], fp32, kind="ExternalOutput")

    with TileContext(nc) as tc:
        with (
            tc.tile_pool(name="mpool", bufs=1) as mpool,
            tc.tile_pool(name="ptpool", bufs=1) as ptpool,
            tc.tile_pool(name="zpool", bufs=1) as zpool,
        ):
            X2 = mpool.tile([P, H, W], fp16, tag="X2")
            PTS = []
            for i, (mr, mc) in sorted(_pt_dims().items()):
                PTi = ptpool.tile(
                    [P, mr, G * mc], fp16, tag=f"PT{i}", name=f"PT{i}"
                )
                PTS.append(PTi)
            TMP = zpool.tile([P, N - 1, G], fp16, tag="TMP")
            Rdf = zpool.tile([P, N, G], fp32, tag="Rdf")
            # fixed per-step scratch (same-engine reuse needs no semaphores)
            R16 = zpool.tile([P, G], fp16, tag="R16")
            QF = zpool.tile([P, G], fp16, tag="QF")
            scol = zpool.tile([P, N, G], fp16, tag="scol")
            ZF = zpool.tile([P, W], fp32, tag="ZF")
            S2 = zpool.tile([P, G, 1], fp32, tag="S2")
            RS = zpool.tile([P, G, 1], fp32, tag="RS")
            WV = zpool.tile([P, G, N], fp32, tag="WV")

            # Row-chunk DMAs aligned exactly to step-0's band boundaries:
            # band d of step 0 touches only rows < its r1, so compute can
            # start as soon as the first ~10-row chunk lands and the rest
            # of the load pipelines behind the step-0 band updates. One
            # touch per chunk so each instruction carries one sem wait.
            rsplit = [0] + [r1 for (_, r0, r1, c0, c1) in _bands(1)]
            nchunks = len(rsplit) - 1
            for ci in range(nchunks):
                nc.sync.dma_start(
                    out=X2[:, rsplit[ci]:rsplit[ci + 1], :],
                    in_=sig16[:, rsplit[ci]:rsplit[ci + 1], :],
                )

            def _touch(ci):
                r = rsplit[ci]
                nc.vector.tensor_copy(X2[:, r, 0:2], X2[:, r, 0:2])

            # ---- forward: symmetric GE on interleaved lower + rhs row ----
            for k in range(N - 1):
                lo = k + 1
                piv = X2[:, k, G * k:G * k + G]
                if k == 0:
                    _touch(0)
                # fp16 output rounds the internally-fp32 reciprocal — same
                # numerics as recip-to-fp32 followed by a cast (validated
                # against the fp32-reciprocal numpy sim: rel err 8.1e-3).
                with nc.allow_low_precision(
                    reason="fp16 recip == fp32 recip + fp16 cast"
                ):
                    nc.vector.reciprocal(R16[:, :], piv)

                def _scol(a, b):
                    # scol[r, g] = X2[lo+r, G*k+g] * R16[g], rows a..b-1
                    nc.vector.tensor_tensor(
                        out=scol[:, a - lo:b - lo, :],
                        in0=X2[:, a:b, G * k:G * k + G],
                        in1=R16[:, :].unsqueeze(1).broadcast_to(
                            [P, b - a, G]
                        ),
                        op=Alu.mult,
                    )

                if k > 0:
                    _scol(lo, H)
                else:
                    touched = 1  # chunk 0 touched before the reciprocal
                # All band multiplies first, then all subtracts: the
                # subs never write column k or scol, so each dependent
                # mult->sub pair is separated by independent work.
                for (pi, r0, r1, c0, c1) in _bands(lo):
                    if k == 0:
                        # touch chunks as step-0 bands first reach them
                        while touched < nchunks and r1 > rsplit[touched]:
                            _touch(touched)
                            touched += 1
                        _scol(r0, r1)
                    nr = r1 - r0
                    ncc = c1 - c0
                    PT = PTS[pi]
                    # PT[r, j, g] = scol[r, g] * X2[j(as row), G*k+g]
                    nc.vector.tensor_tensor(
                        out=PT[:, 0:nr, 0:G * ncc].rearrange(
                            "p a (b c) -> p a b c", c=G
                        ),
                        in0=scol[:, r0 - lo:r1 - lo, :]
                        .unsqueeze(2)
                        .broadcast_to([P, nr, ncc, G]),
                        in1=X2[:, c0:c1, G * k:G * k + G]
                        .unsqueeze(1)
                        .broadcast_to([P, nr, ncc, G]),
                        op=Alu.mult,
                    )
                for (pi, r0, r1, c0, c1) in _bands(lo):
                    nr = r1 - r0
                    ncc = c1 - c0
                    PT = PTS[pi]
                    nc.vector.tensor_tensor(
                        out=X2[:, r0:r1, G * c0:G * c1],
                        in0=X2[:, r0:r1, G * c0:G * c1],
                        in1=PT[:, 0:nr, 0:G * ncc],
                        op=Alu.subtract,
                    )

            # ---- diag reciprocals (one op) ----
            diagv = X2[:, 0:N, :].rearrange(
                "p a (b c) -> p (a b) c", c=G
            )[:, 0:N * N:N + 1, :]
            nc.vector.reciprocal(Rdf[:, :, :], diagv)

            # ---- back-substitution on rhs row (yhat recurrence) ----
            # yhat[0:i] -= row_i[0:i] * (Rd_i * yhat_i)
            for i in range(N - 1, 0, -1):
                q = QF
                with nc.allow_low_precision(reason="fp16 backsolve factor"):
                    nc.vector.tensor_tensor(
                        out=q[:, :],
                        in0=X2[:, H - 1, G * i:G * i + G],
                        in1=Rdf[:, i, :],
                        op=Alu.mult,
                    )
                nc.vector.tensor_tensor(
                    out=TMP[:, 0:i, :],
                    in0=X2[:, i, 0:G * i].rearrange("p (a b) -> p a b", b=G),
                    in1=q[:, :].unsqueeze(1).broadcast_to([P, i, G]),
                    op=Alu.mult,
                )
                nc.vector.tensor_tensor(
                    out=X2[:, H - 1, 0:G * i],
                    in0=X2[:, H - 1, 0:G * i],
                    in1=TMP[:, 0:i, :].rearrange("p a b -> p (a b)"),
                    op=Alu.subtract,
                )

            # ---- z = yhat * Rd (sign cancels in normalization) ----
            nc.vector.tensor_tensor(
                out=ZF[:, :],
                in0=X2[:, H - 1, :],
                in1=Rdf[:, :, :].rearrange("p a b -> p (a b)"),
                op=Alu.mult,
            )
            # per-matrix sums: view [P, G, 64] (stride 1 over g) and
            # reduce the innermost j axis
            nc.vector.tensor_reduce(
                out=S2[:, :, :],
                in_=ZF[:, :].rearrange("p (a b) -> p b a", b=G),
                axis=mybir.AxisListType.X,
                op=Alu.add,
            )
            nc.vector.reciprocal(RS[:, :, :], S2[:, :, :])
            # w = z / sum, de-interleaved to [g, n]
            nc.vector.tensor_tensor(
                out=WV[:, :, :],
                in0=ZF[:, :].rearrange("p (a b) -> p b a", b=G),
                in1=RS[:, :, :].broadcast_to([P, G, N]),
                op=Alu.mult,
            )
            nc.sync.dma_start(out=wout[:, :, :], in_=WV[:, :, :])

    return nc


def _prep_core(shard):
    """[1024, 64, 64] fp32 -> [P, H, W] fp16 interleaved."""
    til = shard.reshape(G, P, N, N).astype(np.float16)
    arr = np.empty((P, H, W), dtype=np.float16)
    arr[:, :N, :].reshape(P, N, N, G)[...] = til.transpose(1, 2, 3, 0)
    arr[:, N, :] = np.float16(1.0)
    return arr


def _unprep_core(warr):
    """[P, G, N] fp32 -> [1024, 64] fp32."""
    return warr.transpose(1, 0, 2).reshape(BPC, N)


def kernel(sigma: np.ndarray) -> np.ndarray:
    global LAST_EXEC_NS
    import time

    from concourse.bass_utils import run_bass_kernel_spmd

    if "nc" not in _CACHE:
        _CACHE["nc"] = _build_program()
    nc = _CACHE["nc"]

    sigma = np.ascontiguousarray(sigma, dtype=np.float32)
    shards = sigma.reshape(NCORES, BPC, N, N)
    in_maps = [{"sig16": _prep_core(shards[i])} for i in range(NCORES)]

    res = run_bass_kernel_spmd(nc, in_maps, core_ids=list(range(NCORES)))

    if os.environ.get("BASS_KERNEL_TIME", "0") == "1":
        # On-device NEFF time via neuron-profile when available; wall time
        # of a warm run otherwise.
        exec_ns = None
        try:
            exec_ns, _ = profile_exec_ns(sigma)
        except Exception:
            exec_ns = None
        if exec_ns is None:
            t0 = time.perf_counter()
            res = run_bass_kernel_spmd(
                nc, in_maps, core_ids=list(range(NCORES))
            )
            exec_ns = int((time.perf_counter() - t0) * 1e9)
        LAST_EXEC_NS = exec_ns

    out = np.concatenate(
        [_unprep_core(res.results[i]["w"]) for i in range(NCORES)], axis=0
    )
    return out.reshape(B, N, 1).astype(np.float32)


def profile_exec_ns(sigma: np.ndarray, tmpdir: str | None = None):
    """Run once with NTFF tracing; returns (exec_time_ns, output)."""
    from concourse.bass_utils import run_bass_kernel_spmd

    if "nc" not in _CACHE:
        _CACHE["nc"] = _build_program()
    nc = _CACHE["nc"]
    sigma = np.ascontiguousarray(sigma, dtype=np.float32)
    shards = sigma.reshape(NCORES, BPC, N, N)
    in_maps = [{"sig16": _prep_core(shards[i])} for i in range(NCORES)]
    res = run_bass_kernel_spmd(
        nc, in_maps, core_ids=list(range(NCORES)), trace=True, tmpdir=tmpdir
    )
    out = np.concatenate(
        [_unprep_core(res.results[i]["w"]) for i in range(NCORES)], axis=0
    )
    return res.exec_time_ns, out.reshape(B, N, 1).astype(np.float32)


# revision 22
# speedup vs baseline: 1.0161x; 1.0002x over previous
"""Batched min-variance weights kernel for Trainium2.

w = S^-1 1 / (1^T S^-1 1) for 8192 SPD 64x64 matrices, data-parallel over
8 cores (1024 matrices each). Measured ~0.61 ms on-device vs the 2.19 ms
v2 baseline.

Design:
- Full-interleave fp16 layout: each SBUF partition lane holds its 8
  matrices element-interleaved — element (i, 8*j+g) is matrix g's (i,j).
  With 2-byte elements every rank-1-update operand has innermost stride
  +1 over the G=8 group and 16-byte-aligned row starts, so the DVE's
  2x_1P fp16 perf mode engages on both the outer-product multiply and
  the subtract (hardware-verified 0.52 ns/elem vs 1.04 for fp32), AND
  each elimination step needs only ~O(bands) instructions for all 1024
  matrices (the v2 baseline issued ~4500 tiny per-tile ops and was ~80%
  per-instruction overhead).
- fp32->fp16 conversion + interleaving happen on HOST (numpy): the
  device reads a pre-interleaved fp16 tensor (on-device CAST is slow,
  and this also halves the host->device transfer).
- Symmetric elimination updates a staircase of row-bands covering the
  lower triangle + the bordered rhs row (row 64); the band count per
  step (up to ~7) minimizes measured fixed-cost (152 ns/op) vs streamed
  area. Overspill above the diagonal is harmless (dead data). Input DMA
  is chunked to step-0's first bands so compute fully hides the load.
- Back-substitution: diagonal reciprocals in ONE op after the forward
  pass, then per step a tiny q = Rd_i*yhat_i factor and 2 folded ops;
  normalization and de-interleave fuse into one op.
- Precision: fp16 storage, reciprocals rounded to fp16 (numpy-validated;
  measured rel err 9.9e-3 full-batch vs the 2e-2 tolerance).
- GpSimd deliberately unused: concurrent pool ops slow DVE 4.3x via the
  shared SBUF port (hardware-measured), so DVE-only is strictly faster.
"""

import os

import numpy as np

B = 8192
N = 64
H = N + 1          # 64 matrix rows + rhs row
G = 8              # matrices interleaved per lane
W = G * N          # interleaved row width (512)
NCORES = 8
BPC = B // NCORES
P = 128

_CACHE = {}
LAST_EXEC_NS = None


def _patch_tail_drain():
    import concourse.mybir as mybir
    import concourse.tile as tile_mod
    from concourse.bass import SemaphoreHandle
    from concourse.vector_clock import ScopedClock

    if getattr(tile_mod.TileContext, "_drain_split_patched", False):
        return

    def _drain_and_barrier(self, tick_clock, wait_clock):
        drain_inst = self.nc.sync.drain()
        wait_clock.add_sem_waits(
            drain_inst.ins, ScopedClock({None: tick_clock.global_clock})
        )
        si = drain_inst.ins.sync_info
        if si is not None and len(si.on_wait) > 1:
            waits = list(si.on_wait)
            drain_inst.ins.sync_info = mybir.SyncInfo(
                on_wait=[waits[0]], on_update=list(si.on_update)
            )
            for w in waits[1:]:
                self.nc.sync.wait_ge(
                    SemaphoreHandle(w.ant_name, w.id), w.wait_value
                )
        self.nc.all_engine_barrier()
        assert self.sems is not None
        popped = self.nc._tile_sem_poison_stack.pop()
        assert popped is self._sem_poison
        self.nc.clear_and_free_semaphores(list(self.sems.allocated().values()))
        self.nc.all_engine_barrier()

    tile_mod.TileContext._drain_and_barrier = _drain_and_barrier
    tile_mod.TileContext._drain_split_patched = True


_BAND_FIXED_NS = 310.0        # measured cost of an extra (mult+sub) pair
_BAND_RATE_NS = G * 2 * 0.52  # ns per staircase area-unit (both passes)


def _bands(lo):
    """Row-bands covering {(i,j): lo<=i<=64, lo<=j<=min(i,63)} (row 64 is
    the bordered rhs row, needing cols lo..63). Band rows [r0,r1) update
    cols [lo, c1); overspill above the diagonal is dead data. The band
    count minimizes stream-vs-instruction-overhead cost; band index
    selects the PT scratch tile (sized for the worst case)."""
    best = None
    for nb in range(1, 10):
        rows = H - lo
        bounds = [lo + (rows * i) // nb for i in range(nb + 1)]
        bl = []
        area = 0
        for i in range(nb):
            r0, r1 = bounds[i], bounds[i + 1]
            if r1 <= r0:
                continue
            c1 = min(r1 - 1, N - 1) + 1
            area += (r1 - r0) * (c1 - lo)
            bl.append((r0, r1, lo, c1))
        cost = area * _BAND_RATE_NS + len(bl) * _BAND_FIXED_NS
        if best is None or cost < best[0]:
            best = (cost, bl)
    return [(i, r0, r1, c0, c1) for i, (r0, r1, c0, c1) in enumerate(best[1])]


def _pt_dims():
    """Max (rows, cols) per band index across all steps."""
    md = {}
    for k in range(N - 1):
        for (i, r0, r1, c0, c1) in _bands(k + 1):
            mr, mc = md.get(i, (0, 0))
            md[i] = (max(mr, r1 - r0), max(mc, c1 - c0))
    return md


def _sc2(t):
    return t


def _build_program():
    import concourse.bass as bass
    import concourse.mybir as mybir
    from concourse.tile import TileContext

    _patch_tail_drain()

    fp32 = mybir.dt.float32
    fp16 = mybir.dt.float16
    Alu = mybir.AluOpType

    nc = bass.Bass()
    sig16 = nc.dram_tensor("sig16", [P, H, W], fp16, kind="ExternalInput")
    wout = nc.dram_tensor("w", [P, G, N], fp32, kind="ExternalOutput")

    with TileContext(nc) as tc:
        with (
            tc.tile_pool(name="mpool", bufs=1) as mpool,
            tc.tile_pool(name="ptpool", bufs=1) as ptpool,
            tc.tile_pool(name="zpool", bufs=1) as zpool,
        ):
            X2 = mpool.tile([P, H, W], fp16, tag="X2")
            PTS = []
            for i, (mr, mc) in sorted(_pt_dims().items()):
                PTi = ptpool.tile(
                    [P, mr, G * mc], fp16, tag=f"PT{i}", name=f"PT{i}"
                )
                PTS.append(PTi)
            TMP = zpool.tile([P, N - 1, G], fp16, tag="TMP")
            Rdf = zpool.tile([P, N, G], fp32, tag="Rdf")
            # fixed per-step scratch (same-engine reuse needs no semaphores)
            R16 = zpool.tile([P, G], fp16, tag="R16")
            R16P = zpool.tile([P, G], fp16, tag="R16P")
            scolP = zpool.tile([P, 16, G], fp16, tag="scolP")
            QF = zpool.tile([P, G], fp16, tag="QF")
            scol = zpool.tile([P, N, G], fp16, tag="scol")
            ZF = zpool.tile([P, W], fp32, tag="ZF")
            S2 = zpool.tile([P, G, 1], fp32, tag="S2")
            RS = zpool.tile([P, G, 1], fp32, tag="RS")
            WV = zpool.tile([P, G, N], fp32, tag="WV")

            # Row-chunk DMAs aligned exactly to step-0's band boundaries:
            # band d of step 0 touches only rows < its r1, so compute can
            # start as soon as the first ~10-row chunk lands and the rest
            # of the load pipelines behind the step-0 band updates. One
            # touch per chunk so each instruction carries one sem wait.
            rsplit = [0] + [r1 for (_, r0, r1, c0, c1) in _bands(1)]
            nchunks = len(rsplit) - 1
            for ci in range(nchunks):
                nc.sync.dma_start(
                    out=X2[:, rsplit[ci]:rsplit[ci + 1], :],
                    in_=sig16[:, rsplit[ci]:rsplit[ci + 1], :],
                )

            def _touch(ci):
                r = rsplit[ci]
                nc.vector.tensor_copy(X2[:, r, 0:2], X2[:, r, 0:2])

            # ---- forward: symmetric GE on interleaved lower + rhs row ----
            for k in range(N - 1):
                lo = k + 1
                piv = X2[:, k, G * k:G * k + G]
                if k == 0:
                    _touch(0)
                # fp16 output rounds the internally-fp32 reciprocal — same
                # numerics as recip-to-fp32 followed by a cast (validated
                # against the fp32-reciprocal numpy sim: rel err 8.1e-3).
                with nc.allow_low_precision(
                    reason="fp16 recip == fp32 recip + fp16 cast"
                ):
                    nc.vector.reciprocal(R16[:, :], piv)

                def _scol(a, b):
                    # scol[r, g] = X2[lo+r, G*k+g] * R16[g], rows a..b-1
                    nc.vector.tensor_tensor(
                        out=scol[:, a - lo:b - lo, :],
                        in0=X2[:, a:b, G * k:G * k + G],
                        in1=R16[:, :].unsqueeze(1).broadcast_to(
                            [P, b - a, G]
                        ),
                        op=Alu.mult,
                    )

                def _mult(kk, pi, r0, r1, c0, c1, sc):
                    nr = r1 - r0
                    ncc = c1 - c0
                    PT = PTS[pi]
                    # PT[r, j, g] = sc[r, g] * X2[j(as row), G*kk+g]
                    nc.vector.tensor_tensor(
                        out=PT[:, 0:nr, 0:G * ncc].rearrange(
                            "p a (b c) -> p a b c", c=G
                        ),
                        in0=sc[:, r0 - kk - 1:r1 - kk - 1, :]
                        .unsqueeze(2)
                        .broadcast_to([P, nr, ncc, G]),
                        in1=X2[:, c0:c1, G * kk:G * kk + G]
                        .unsqueeze(1)
                        .broadcast_to([P, nr, ncc, G]),
                        op=Alu.mult,
                    )

                def _sub(pi, r0, r1, c0, c1):
                    nr = r1 - r0
                    ncc = c1 - c0
                    PT = PTS[pi]
                    nc.vector.tensor_tensor(
                        out=X2[:, r0:r1, G * c0:G * c1],
                        in0=X2[:, r0:r1, G * c0:G * c1],
                        in1=PT[:, 0:nr, 0:G * ncc],
                        op=Alu.subtract,
                    )

                if k == 0:
                    # Interleave mult+sub per band so step-0 compute
                    # pipelines against the chunked load; after band 1,
                    # prepull step-1's first band (its rows/cols are fully
                    # updated by step-0 bands 0-1) to fill the chunk wait.
                    touched = 1
                    for (pi, r0, r1, c0, c1) in _bands(lo):
                        while touched < nchunks and r1 > rsplit[touched]:
                            _touch(touched)
                            touched += 1
                        _scol(r0, r1)
                        _mult(0, pi, r0, r1, c0, c1, scol)
                        _sub(pi, r0, r1, c0, c1)
                        if pi == 1:
                            (p1, s0, s1, d0, d1) = _bands(2)[0]
                            assert s1 <= rsplit[2] and d1 <= rsplit[2]
                            with nc.allow_low_precision(reason="fp16 recip"):
                                nc.vector.reciprocal(
                                    R16P[:, :], X2[:, 1, G:2 * G]
                                )
                            nc.vector.tensor_tensor(
                                out=scolP[:, 0:s1 - 2, :],
                                in0=X2[:, 2:s1, G:2 * G],
                                in1=R16P[:, :]
                                .unsqueeze(1)
                                .broadcast_to([P, s1 - 2, G]),
                                op=Alu.mult,
                            )
                            _mult(1, p1, s0, s1, d0, d1, _sc2(scolP))
                            _sub(p1, s0, s1, d0, d1)
                elif k == 1:
                    _scol(lo, H)
                    bands = [b for b in _bands(lo) if b[0] != 0]
                    for (pi, r0, r1, c0, c1) in bands:
                        _mult(k, pi, r0, r1, c0, c1, scol)
                    for (pi, r0, r1, c0, c1) in bands:
                        _sub(pi, r0, r1, c0, c1)
                else:
                    _scol(lo, H)
                    for (pi, r0, r1, c0, c1) in _bands(lo):
                        _mult(k, pi, r0, r1, c0, c1, scol)
                    for (pi, r0, r1, c0, c1) in _bands(lo):
                        _sub(pi, r0, r1, c0, c1)

            # ---- diag reciprocals (one op) ----
            diagv = X2[:, 0:N, :].rearrange(
                "p a (b c) -> p (a b) c", c=G
            )[:, 0:N * N:N + 1, :]
            nc.vector.reciprocal(Rdf[:, :, :], diagv)

            # ---- back-substitution on rhs row (yhat recurrence) ----
            # yhat[0:i] -= row_i[0:i] * (Rd_i * yhat_i)
            for i in range(N - 1, 0, -1):
                q = QF
                with nc.allow_low_precision(reason="fp16 backsolve factor"):
                    nc.vector.tensor_tensor(
                        out=q[:, :],
                        in0=X2[:, H - 1, G * i:G * i + G],
                        in1=Rdf[:, i, :],
                        op=Alu.mult,
                    )
                nc.vector.tensor_tensor(
                    out=TMP[:, 0:i, :],
                    in0=X2[:, i, 0:G * i].rearrange("p (a b) -> p a b", b=G),
                    in1=q[:, :].unsqueeze(1).broadcast_to([P, i, G]),
                    op=Alu.mult,
                )
                nc.vector.tensor_tensor(
                    out=X2[:, H - 1, 0:G * i],
                    in0=X2[:, H - 1, 0:G * i],
                    in1=TMP[:, 0:i, :].rearrange("p a b -> p (a b)"),
                    op=Alu.subtract,
                )

            # ---- z = yhat * Rd (sign cancels in normalization) ----
            nc.vector.tensor_tensor(
                out=ZF[:, :],
                in0=X2[:, H - 1, :],
                in1=Rdf[:, :, :].rearrange("p a b -> p (a b)"),
                op=Alu.mult,
            )
            # per-matrix sums: view [P, G, 64] (stride 1 over g) and
            # reduce the innermost j axis
            nc.vector.tensor_reduce(
                out=S2[:, :, :],
                in_=ZF[:, :].rearrange("p (a b) -> p b a", b=G),
                axis=mybir.AxisListType.X,
                op=Alu.add,
            )
            nc.vector.reciprocal(RS[:, :, :], S2[:, :, :])
            # w = z / sum, de-interleaved to [g, n]
            nc.vector.tensor_tensor(
                out=WV[:, :, :],
                in0=ZF[:, :].rearrange("p (a b) -> p b a", b=G),
                in1=RS[:, :, :].broadcast_to([P, G, N]),
                op=Alu.mult,
            )
            nc.sync.dma_start(out=wout[:, :, :], in_=WV[:, :, :])

    return nc


def _prep_core(shard):
    """[1024, 64, 64] fp32 -> [P, H, W] fp16 interleaved."""
    til = shard.reshape(G, P, N, N).astype(np.float16)
    arr = np.empty((P, H, W), dtype=np.float16)
    arr[:, :N, :].reshape(P, N, N, G)[...] = til.transpose(1, 2, 3, 0)
    arr[:, N, :] = np.float16(1.0)
    return arr


def _unprep_core(warr):
    """[P, G, N] fp32 -> [1024, 64] fp32."""
    return warr.transpose(1, 0, 2).reshape(BPC, N)


def kernel(sigma: np.ndarray) -> np.ndarray:
    global LAST_EXEC_NS
    import time

    from concourse.bass_utils import run_bass_kernel_spmd

    if "nc" not in _CACHE:
        _CACHE["nc"] = _build_program()
    nc = _CACHE["nc"]

    sigma = np.ascontiguousarray(sigma, dtype=np.float32)
    shards = sigma.reshape(NCORES, BPC, N, N)
    in_maps = [{"sig16": _prep_core(shards[i])} for i in range(NCORES)]

    res = run_bass_kernel_spmd(nc, in_maps, core_ids=list(range(NCORES)))

    if os.environ.get("BASS_KERNEL_TIME", "0") == "1":
        # On-device NEFF time via neuron-profile when available; wall time
        # of a warm run otherwise.
        exec_ns = None
        try:
            exec_ns, _ = profile_exec_ns(sigma)
        except Exception:
            exec_ns = None
        if exec_ns is None:
            t0 = time.perf_counter()
            res = run_bass_kernel_spmd(
                nc, in_maps, core_ids=list(range(NCORES))
            )
            exec_ns = int((time.perf_counter() - t0) * 1e9)
        LAST_EXEC_NS = exec_ns

    out = np.concatenate(
        [_unprep_core(res.results[i]["w"]) for i in range(NCORES)], axis=0
    )
    return out.reshape(B, N, 1).astype(np.float32)


def profile_exec_ns(sigma: np.ndarray, tmpdir: str | None = None):
    """Run once with NTFF tracing; returns (exec_time_ns, output)."""
    from concourse.bass_utils import run_bass_kernel_spmd

    if "nc" not in _CACHE:
        _CACHE["nc"] = _build_program()
    nc = _CACHE["nc"]
    sigma = np.ascontiguousarray(sigma, dtype=np.float32)
    shards = sigma.reshape(NCORES, BPC, N, N)
    in_maps = [{"sig16": _prep_core(shards[i])} for i in range(NCORES)]
    res = run_bass_kernel_spmd(
        nc, in_maps, core_ids=list(range(NCORES)), trace=True, tmpdir=tmpdir
    )
    out = np.concatenate(
        [_unprep_core(res.results[i]["w"]) for i in range(NCORES)], axis=0
    )
    return res.exec_time_ns, out.reshape(B, N, 1).astype(np.float32)
